# revision 1
# baseline (speedup 1.0000x reference)
"""Cy2MixerBlock (nn_Cy2MixerBlock_6700148982551) Trainium2 Bass kernel.

Per (b,t) slice (N=325 nodes, D=F=128 features), fully independent:
  natural  = (node partitions [3 tiles 128/128/69], feature free)
  f-major  = (feature partitions [exactly 128], node free [325])
Matmuls contract over partitions; LN stats reduce over free dim in natural
layout; transposes via PE identity-matmul; SGU conv slides over the FEATURE
axis -> implemented as 9 accumulated matmuls whose stationary operand is a
column-shifted view of a zero-padded vn tile.
"""

import numpy as np
import ml_dtypes
from contextlib import ExitStack

import concourse.bass as bass
import concourse.bacc as bacc
import concourse.mybir as mybir
import concourse.tile as tile
from concourse import bass_utils
from concourse.masks import make_identity

F32 = mybir.dt.float32
F32R = mybir.dt.float32r
BF16 = mybir.dt.bfloat16
AF = mybir.ActivationFunctionType
ALU = mybir.AluOpType

B, T, N, D = 32, 12, 325, 128
F = D
NCORES = 8
BPC = B // NCORES  # batches per core
PKS = (128, 128, 69)  # node-tile partition sizes
NP = 326  # node free dim padded even (fp32r matmul dst must be even-sized)
EPS = 1e-5


# ---------------------------------------------------------------- host folds
def host_fold(inp):
    """Exact algebraic folds (numpy fp32). Returns dict of device arrays +
    flags for the general (nonzero-bias) paths."""
    g = inp["norm_g"].astype(np.float32)
    bn = inp["norm_b"].astype(np.float32)
    Wqkv = inp["Wqkv"].astype(np.float32)
    bqkv = inp["bqkv"].astype(np.float32)
    # LN1 affine folded into all consumers of xn
    Wqkv_f = g[:, None] * Wqkv
    bqkv_f = bqkv + bn @ Wqkv
    W1e = inp["W1"].astype(np.float32) + inp["Wl"].astype(np.float32) @ inp["aff1_W"].astype(np.float32)
    b1e = inp["b1"].astype(np.float32) + inp["bl"].astype(np.float32) @ inp["aff1_W"].astype(np.float32) + inp["aff1_b"].astype(np.float32)
    W1f = g[:, None] * W1e
    b1f = b1e + bn @ W1e

    Wq = Wqkv_f[:, 0:F]
    Wk = Wqkv_f[:, F:2 * F]
    Wv = Wqkv_f[:, 2 * F:3 * F]
    A = (Wq @ Wk.T)  # logits = xn A xn^T
    bq = bqkv_f[0:F]
    bk = bqkv_f[F:2 * F]
    bv = bqkv_f[2 * F:3 * F]

    Wo = inp["Wo"].astype(np.float32)
    bo2 = inp["bo"].astype(np.float32) + bv @ Wo  # v-bias exact fold through softmax

    conv_w = inp["conv_w"].astype(np.float32)  # (N, N, 1, K)
    # conv rhs: wdtT[kw][k] = conv_w[:, k*128:+pk, 0, kw].T  -> (pk, 325), pad to 128
    wdtT = np.zeros((3, 3, 128, NP), np.float32)
    for kw in range(3):
        for k in range(3):
            pk = PKS[k]
            wdtT[kw, k, :pk, :N] = conv_w[:, k * 128:k * 128 + pk, 0, kw].T

    d = dict(
        A=A.astype(np.float32),
        wv=Wv.astype(np.float32),
        w1f=W1f.astype(np.float32),
        wo_bf=Wo.astype(ml_dtypes.bfloat16),
        w2=inp["W2"].astype(np.float32),
        wdtT=wdtT,
        cb=np.pad(inp["conv_b"].astype(np.float32), (0, NP - N))[None, :],
        ones_row=np.ones((1, 128), np.float32),
        ones_col_bf=np.ones((128, 1), ml_dtypes.bfloat16),
        zpad=np.zeros((128, 2), np.float32),
    )
    flags = dict(
        has_bqk=not (np.all(bq == 0) and np.all(bk == 0)),
        has_b1=not np.all(b1f == 0),
        has_bo2=not np.all(bo2 == 0),
        has_b2=not np.all(inp["b2"] == 0),
        has_sgu=not (np.all(inp["sgu_g"] == 1) and np.all(inp["sgu_b"] == 0)),
    )
    extras = dict(
        b1f=b1f, bo2=bo2, b2=inp["b2"].astype(np.float32),
        bq=bq, bk=bk, Wq=Wq, Wk=Wk,
        sgu_g=inp["sgu_g"].astype(np.float32), sgu_b=inp["sgu_b"].astype(np.float32),
    )
    return d, flags, extras


# ---------------------------------------------------------------- builder
def build_nc(flags, dbg=False, bpc=BPC, loop_n=None):
    nc = bacc.Bacc("TRN2", target_bir_lowering=False, debug=False)

    x_dram = nc.dram_tensor("x_sh", [bpc, T, N, D], F32, kind="ExternalInput")
    o_dram = nc.dram_tensor("out_sh", [bpc, T, N, D], F32, kind="ExternalOutput")
    wd = {}
    for nm, shp in [("A", [D, D]), ("wv", [D, F]), ("w1f", [D, 2 * F]), ("w2", [F, D])]:
        wd[nm] = nc.dram_tensor(nm, shp, F32R, kind="ExternalInput")
    wd["wo_bf"] = nc.dram_tensor("wo_bf", [F, F], BF16, kind="ExternalInput")
    wd["wdtT"] = nc.dram_tensor("wdtT", [3, 3, 128, NP], F32R, kind="ExternalInput")
    wd["cb"] = nc.dram_tensor("cb", [1, NP], F32R, kind="ExternalInput")
    wd["ones_row"] = nc.dram_tensor("ones_row", [1, 128], F32R, kind="ExternalInput")
    wd["ones_col_bf"] = nc.dram_tensor("ones_col_bf", [128, 1], BF16, kind="ExternalInput")
    wd["zpad"] = nc.dram_tensor("zpad", [128, 2], F32R, kind="ExternalInput")
    if flags["has_b1"]:
        wd["b1fT"] = nc.dram_tensor("b1fT", [2 * F, 1], F32, kind="ExternalInput")
        wd["b1vB"] = nc.dram_tensor("b1vB", [128, F], F32, kind="ExternalInput")
    if flags["has_bo2"]:
        wd["bo2T"] = nc.dram_tensor("bo2T", [F, 1], F32, kind="ExternalInput")
    if flags["has_b2"]:
        wd["b2T"] = nc.dram_tensor("b2T", [D, 1], F32, kind="ExternalInput")
    if flags["has_sgu"]:
        wd["gsB"] = nc.dram_tensor("gsB", [128, F], F32, kind="ExternalInput")
        wd["bsB"] = nc.dram_tensor("bsB", [128, F], F32, kind="ExternalInput")
    assert not flags["has_bqk"], "bq/bk general path not implemented"

    dbg_outs = {}
    if dbg:
        for nm, shp in [("d_xnt", [T, D, NP]), ("d_att", [T, F, NP]),
                        ("d_u", [T, F, NP]), ("d_vn", [T, 128, 3, 130]),
                        ("d_sg", [T, F, NP])]:
            dbg_outs[nm] = nc.dram_tensor(nm, [bpc] + shp, F32, kind="ExternalOutput")

    x_ap = x_dram.ap()
    o_ap = o_dram.ap()

    with tile.TileContext(nc) as tc:
        with ExitStack() as ctx:
            const = ctx.enter_context(tc.tile_pool(name="const", bufs=1))
            wrk = ctx.enter_context(tc.tile_pool(name="wrk", bufs=3))
            wrk3 = ctx.enter_context(tc.tile_pool(name="wrk3", bufs=4))
            ps8 = ctx.enter_context(tc.tile_pool(name="ps8", bufs=8, space="PSUM"))

            # ---- constants
            ident = const.tile([128, 128], F32, name="ident")
            make_identity(nc, ident)
            zero_sb = const.tile([128, 1], F32, name="zero_sb")
            nc.vector.memset(zero_sb, 0.0)
            csb = {}
            for nm, t_ in wd.items():
                if nm == "wdtT":
                    csb[nm] = const.tile([128, 3, 3, NP], t_.dtype, name="c_wdtT")
                    nc.sync.dma_start(out=csb[nm], in_=t_.ap().rearrange("a b p n -> p a b n"))
                    continue
                shp = list(t_.shape)
                csb[nm] = const.tile(shp, t_.dtype, name=f"c_{nm}")
                nc.sync.dma_start(out=csb[nm], in_=t_.ap())

            xnt_bufs = []
            for bi in range(6):
                xb = const.tile([128, NP], F32R, name=f"xntbuf{bi}")
                nc.sync.dma_start(out=xb[:, N:NP], in_=csb["zpad"][:, 0:1])
                xnt_bufs.append(xb)

            vnp_bufs = []
            for bi in range(3):
                vb = const.tile([128, 3, 130], F32R, name=f"vnpbuf{bi}")
                if dbg:
                    nc.vector.memset(vb.bitcast(F32), 0.0)
                for k in range(3):
                    nc.sync.dma_start(out=vb[:, k, 0:130:129], in_=csb["zpad"])
                vnp_bufs.append(vb)

            A_sb, wv_sb, w1f_sb, w2_sb = csb["A"], csb["wv"], csb["w1f"], csb["w2"]
            wo_sb, wdtT_sb, cb_sb = csb["wo_bf"], csb["wdtT"], csb["cb"]
            ones_row, ones_col = csb["ones_row"], csb["ones_col_bf"]

            def ts_(k):
                return slice(k * 128, k * 128 + PKS[k])

            MAGIC = 0x5F3759DF

            def rsqrt_cols(pool, var_ap, n):
                """rstd = (var+EPS)^-1/2 on DVE only (bit-trick seed + 2 Newton
                iters, exact to fp32 ulp). var_ap: (128, n) strided view."""
                t_ = pool.tile([128, 4, 3], F32, tag="nw", name="nw")
                t = t_[:, 0, 0:n]
                y = t_[:, 1, 0:n]
                a = t_[:, 2, 0:n]
                c = t_[:, 3, 0:n]
                nc.vector.tensor_scalar_add(t, var_ap, EPS)
                nc.vector.tensor_scalar(
                    out=y.bitcast(mybir.dt.int32), in0=t.bitcast(mybir.dt.int32),
                    scalar1=1, scalar2=None, op0=ALU.logical_shift_right)
                nc.vector.tensor_scalar(
                    out=y.bitcast(mybir.dt.int32), in0=y.bitcast(mybir.dt.int32),
                    scalar1=-1, scalar2=MAGIC, op0=ALU.mult, op1=ALU.add)
                for _ in range(1):
                    nc.vector.tensor_mul(a, y, y)
                    nc.vector.tensor_mul(a, a, t)
                    nc.vector.tensor_scalar(out=c, in0=a, scalar1=-0.5, scalar2=1.5,
                                            op0=ALU.mult, op1=ALU.add)
                    nc.vector.tensor_mul(y, y, c)
                return y

            MAGIC = 0x5F3759DF

            def rsqrt_cols(pool, var_ap, n, eng=None):
                """rstd = (var+EPS)^-1/2, bit-trick seed + 1 Newton iter
                (rel err ~3e-6), on DVE or GPSIMD."""
                e = eng or nc.vector
                t_ = pool.tile([128, 4, 3], F32, tag="nw", name="nw")
                t = t_[:, 0, 0:n]
                y = t_[:, 1, 0:n]
                a = t_[:, 2, 0:n]
                c = t_[:, 3, 0:n]
                e.tensor_scalar_add(t, var_ap, EPS)
                e.tensor_scalar(
                    out=y.bitcast(mybir.dt.int32), in0=t.bitcast(mybir.dt.int32),
                    scalar1=1, scalar2=None, op0=ALU.logical_shift_right)
                e.tensor_scalar(
                    out=y.bitcast(mybir.dt.int32), in0=y.bitcast(mybir.dt.int32),
                    scalar1=-1, scalar2=MAGIC, op0=ALU.mult, op1=ALU.add)
                for _ in range(1):
                    e.tensor_mul(a, y, y)
                    e.tensor_mul(a, a, t)
                    e.tensor_scalar(out=c, in0=a, scalar1=-0.5, scalar2=1.5,
                                    op0=ALU.mult, op1=ALU.add)
                    e.tensor_mul(y, y, c)
                return y

            def s1(b, t):
                st_ = {}
                S[(b, t)] = st_
# ---------- load x natural (3 node tiles)
                xt = wrk3.tile([128, 3, 128], F32, tag="xt", bufs=6, name="xt")
                nc.sync.dma_start(
                    out=xt[:, 0:2, :],
                    in_=x_ap[b, t, 0:256, :].rearrange("(k p) d -> p k d", p=128))
                nc.sync.dma_start(out=xt[:69, 2, :], in_=x_ap[b, t, 256:325, :])

                # ---------- LN1 (natural): batched stats + DVE rsqrt
                mv1 = wrk3.tile([128, 3, 2], F32, tag="mv1", name="mv1")
                nc.vector.memset(mv1[64:128, 2, :], 1.0)
                for k in range(3):
                    pk = PKS[k]
                    stx = wrk3.tile([128, 6], F32, tag="st", name="stx")
                    nc.vector.bn_stats(stx[:pk], xt[:pk, k, :])
                    nc.vector.bn_aggr(mv1[:pk, k, :], stx[:pk])
                rs1 = rsqrt_cols(wrk3, mv1[:, :, 1], 3)
                XnT = xnt_bufs[(b * T + t) % 6]
                tpx = ps8.tile([128, 3, 128], F32, tag="ps", name="tpx")
                for k in range(3):
                    pk = PKS[k]
                    xtmp = wrk3.tile([128, 128], F32, tag="xtmp", name="xtmp")
                    nc.gpsimd.tensor_scalar(
                        out=xtmp[:pk], in0=xt[:pk, k, :], scalar1=mv1[:pk, k, 0:1],
                        scalar2=rs1[:pk, k:k + 1], op0=ALU.subtract, op1=ALU.mult)
                    nc.tensor.transpose(tpx[:, k, :pk], xtmp[:pk, :], ident[:pk, :pk])
                nc.scalar.copy(out=XnT[:, 0:256], in_=tpx[:, 0:2, :])
                nc.scalar.copy(out=XnT[:, 256:325], in_=tpx[:, 2, :69])
                if dbg:
                    nc.sync.dma_start(out=dbg_outs["d_xnt"].ap()[b, t], in_=XnT.bitcast(F32))
                st_["xt"] = xt
                st_["XnT"] = XnT

            def s2a(b, t):
                st_ = S[(b, t)]
                XnT = st_["XnT"]
                Xtr = XnT
# ---------- tiny attention
                g_ps = ps8.tile([128, NP], F32, tag="ps", name="g_ps")
                nc.tensor.matmul(g_ps, A_sb, Xtr, start=True, stop=True)
                G = wrk.tile([128, NP], F32R, tag="G", name="G")
                nc.scalar.copy(G, g_ps)
                E = wrk.tile([128, 3, NP], BF16, tag="E", bufs=3, name="E")
                for k in range(3):
                    pk = PKS[k]
                    lt = ps8.tile([128, NP], F32, tag="ps", name="lt")
                    nc.tensor.matmul(lt[:pk], XnT[:, ts_(k)],
                                     G, start=True, stop=True)
                    nc.scalar.activation(E[:pk, k, :], lt[:pk], AF.Exp)
                st_["E"] = E

            def s2a2(b, t):
                st_ = S[(b, t)]
                XnT = st_["XnT"]
                E = st_["E"]
                z_ps = ps8.tile([1, NP], F32, tag="ps", name="z_ps")
                for k in range(3):
                    pk = PKS[k]
                    nc.tensor.matmul(z_ps, ones_col[:pk], E[:pk, k, :],
                                     start=(k == 0), stop=(k == 2))
                zr = wrk.tile([1, NP], F32R, tag="zr", name="zr")
                with nc.allow_low_precision(reason="f32r rounding of 1/Z is fine"):
                    nc.vector.reciprocal(zr, z_ps)
                zb_ps = ps8.tile([128, NP], F32, tag="ps", name="zb_ps")
                nc.tensor.matmul(zb_ps, ones_row, zr, start=True, stop=True)
                zb = wrk.tile([128, NP], F32, tag="zb", name="zb")
                nc.scalar.copy(zb, zb_ps)
                vns = wrk.tile([128, 3, 128], BF16, tag="vns", name="vns")
                vp = ps8.tile([128, 3, 128], F32, tag="ps", name="vp")
                for k in range(3):
                    pk = PKS[k]
                    nc.tensor.matmul(vp[:pk, k, :], XnT[:, ts_(k)],
                                     wv_sb, start=True, stop=True)
                    nc.scalar.copy(out=vns[:pk, k, :], in_=vp[:pk, k, :])
                au = ps8.tile([128, NP], F32, tag="ps", name="au")
                for k in range(3):
                    pk = PKS[k]
                    nc.tensor.matmul(au, vns[:pk, k, :], E[:pk, k, :],
                                     start=(k == 0), stop=(k == 2))
                att = wrk.tile([128, NP], BF16, tag="att", bufs=3, name="att")
                nc.vector.tensor_mul(att, au, zb)
                if dbg:
                    dat = wrk.tile([128, NP], F32, tag="dat", name="dat")
                    nc.vector.tensor_copy(dat, att)
                    nc.sync.dma_start(out=dbg_outs["d_att"].ap()[b, t], in_=dat)
                st_["att"] = att

            def s2b(b, t):
                st_ = S[(b, t)]
                XnT = st_["XnT"]
                Xtr = XnT
# ---------- u (f-major) and vv -> LN2 -> vn_pad (natural)
                up = ps8.tile([128, NP], F32, tag="ps", name="up")
                nc.tensor.matmul(up, w1f_sb[:, 0:F], Xtr,
                                 start=True, stop=True)
                u = wrk.tile([128, NP], F32, tag="u", bufs=3, name="u")
                ub = csb["b1fT"][0:F] if flags["has_b1"] else zero_sb
                nc.scalar.activation(u, up, AF.Relu, bias=ub)
                if dbg:
                    nc.sync.dma_start(out=dbg_outs["d_u"].ap()[b, t], in_=u)

                vn_pad = vnp_bufs[(b * T + t) % 3]
                mv2 = wrk3.tile([128, 3, 2], F32, tag="mv1", name="mv2")
                nc.vector.memset(mv2[64:128, 2, :], 1.0)
                vvn = wrk3.tile([128, 3, 128], F32, tag="vvn", name="vvn")
                vvp = ps8.tile([128, 3, 128], F32, tag="ps", name="vvp")
                for k in range(3):
                    pk = PKS[k]
                    nc.tensor.matmul(vvp[:pk, k, :], XnT[:, ts_(k)],
                                     w1f_sb[:, F:2 * F],
                                     start=True, stop=True)
                for k in range(3):
                    pk = PKS[k]
                    if flags["has_b1"]:
                        nc.vector.tensor_add(vvn[:pk, k, :], vvp[:pk, k, :], csb["b1vB"][:pk])
                        nc.vector.tensor_scalar_max(vvn[:pk, k, :], vvn[:pk, k, :], 0.0)
                    else:
                        if k < 2:
                            continue
                        nc.vector.tensor_scalar_max(vvn[:, 0:2, :], vvp[:, 0:2, :], 0.0)
                        nc.vector.tensor_scalar_max(vvn[:69, 2, :], vvp[:69, 2, :], 0.0)
                for k in range(3):
                    pk = PKS[k]
                    st2 = wrk3.tile([128, 6], F32, tag="st", name="st2")
                    nc.vector.bn_stats(st2[:pk], vvn[:pk, k, :])
                    nc.vector.bn_aggr(mv2[:pk, k, :], st2[:pk])
                rs2 = rsqrt_cols(wrk3, mv2[:, :, 1], 3)
                for k in range(3):
                    pk = PKS[k]
                    nc.gpsimd.tensor_scalar(
                        out=vn_pad[:pk, k, 1:129], in0=vvn[:pk, k, :],
                        scalar1=mv2[:pk, k, 0:1], scalar2=rs2[:pk, k:k + 1],
                        op0=ALU.subtract, op1=ALU.mult)
                    if flags["has_sgu"]:
                        nc.vector.tensor_mul(vn_pad[:pk, k, 1:129], vn_pad[:pk, k, 1:129], csb["gsB"][:pk])
                        nc.vector.tensor_add(vn_pad[:pk, k, 1:129], vn_pad[:pk, k, 1:129], csb["bsB"][:pk])
                if dbg:
                    nc.sync.dma_start(out=dbg_outs["d_vn"].ap()[b, t], in_=vn_pad.bitcast(F32))
                st_["u"] = u
                st_["vn_pad"] = vn_pad

            def s3(b, t):
                st_ = S[(b, t)]
                xt = st_["xt"]
                att = st_["att"]
                u = st_["u"]
                vn_pad = st_["vn_pad"]
# ---------- conv over f (9 MMs) + conv bias rank-1 + gate
                co = ps8.tile([128, NP], F32, tag="ps", name="co")
                first = True
                for kw in range(3):
                    for k in range(3):
                        pk = PKS[k]
                        nc.tensor.matmul(co, vn_pad[:pk, k, kw:kw + 128],
                                         wdtT_sb[:pk, kw, k, :],
                                         start=first, stop=False)
                        first = False
                nc.tensor.matmul(co, ones_row, cb_sb,
                                 start=False, stop=False)
                nc.tensor.matmul(co, wo_sb, att, start=False, stop=True)
                # ---------- sg = (co + bo2) * u ; out = relu(W2^T sg) ; +residual
                sg = wrk.tile([128, NP], F32R, tag="sg", name="sg")
                bo_s = csb["bo2T"] if flags["has_bo2"] else 0.0
                nc.vector.scalar_tensor_tensor(out=sg, in0=co, scalar=bo_s, in1=u,
                                               op0=ALU.add, op1=ALU.mult)
                if dbg:
                    nc.sync.dma_start(out=dbg_outs["d_sg"].ap()[b, t], in_=sg.bitcast(F32))
                op_ = ps8.tile([128, NP], F32, tag="ps", name="op_")
                nc.tensor.matmul(op_, w2_sb, sg,
                                 start=True, stop=True)
                ot = wrk.tile([128, NP], F32, tag="ot", name="ot")
                b2b = csb["b2T"] if flags["has_b2"] else zero_sb
                nc.scalar.activation(ot, op_, AF.Relu, bias=b2b)
                onat = wrk.tile([128, 3, 128], F32, tag="onat", name="onat")
                tpo = ps8.tile([128, 3, 128], F32, tag="ps", name="tpo")
                for k in range(3):
                    pk = PKS[k]
                    nc.tensor.transpose(tpo[:pk, k, :], ot[:, ts_(k)], ident)
                nc.vector.tensor_add(onat[:, 0:2, :], tpo[:, 0:2, :], xt[:, 0:2, :])
                nc.vector.tensor_add(onat[:69, 2, :], tpo[:69, 2, :], xt[:69, 2, :])
                nc.scalar.dma_start(
                    out=o_ap[b, t, 0:256, :].rearrange("(k p) d -> p k d", p=128),
                    in_=onat[:, 0:2, :])
                nc.scalar.dma_start(out=o_ap[b, t, 256:325, :], in_=onat[:69, 2, :])

            seq = [(b, t) for b in range(bpc) for t in range(T)]

            def emit_all():
                S.clear()
                for i in range(len(seq) + 4):
                    if i < len(seq):
                        s1(*seq[i])
                    if 1 <= i < len(seq) + 1:
                        s2a(*seq[i - 1])
                    if 2 <= i < len(seq) + 2:
                        s2a2(*seq[i - 2])
                    if 3 <= i < len(seq) + 3:
                        s2b(*seq[i - 3])
                    if 4 <= i < len(seq) + 4:
                        s3(*seq[i - 4])
                        del S[seq[i - 4]]

            S = {}
            if loop_n:
                with tc.For_i(0, loop_n, 1):
                    emit_all()
            else:
                emit_all()

    nc.compile()
    return nc


# ---------------------------------------------------------------- runner
def make_in_maps(inputs, dev, flags, extras):
    x = np.ascontiguousarray(inputs["x"], dtype=np.float32)
    maps = []
    for c in range(NCORES):
        m = dict(dev)
        if flags["has_b1"]:
            m["b1fT"] = extras["b1f"][:, None].astype(np.float32)
            m["b1vB"] = np.broadcast_to(extras["b1f"][None, F:2 * F], (128, F)).astype(np.float32).copy()
        if flags["has_bo2"]:
            m["bo2T"] = extras["bo2"][:, None].astype(np.float32)
        if flags["has_b2"]:
            m["b2T"] = extras["b2"][:, None].astype(np.float32)
        if flags["has_sgu"]:
            m["gsB"] = np.broadcast_to(extras["sgu_g"][None, :], (128, F)).astype(np.float32).copy()
            m["bsB"] = np.broadcast_to(extras["sgu_b"][None, :], (128, F)).astype(np.float32).copy()
        m["x_sh"] = x[c * BPC:(c + 1) * BPC]
        maps.append(m)
    return maps


_NC_CACHE = {}


def kernel(**inputs):
    dev, flags, extras = host_fold(inputs)
    key = tuple(sorted(flags.items()))
    if key not in _NC_CACHE:
        _NC_CACHE[key] = build_nc(flags, dbg=False)
    nc = _NC_CACHE[key]
    in_maps = make_in_maps(inputs, dev, flags, extras)
    res = bass_utils.run_bass_kernel_spmd(nc, in_maps, core_ids=list(range(NCORES)))
    out = np.concatenate([res.results[c]["out_sh"] for c in range(NCORES)], axis=0)
    return np.ascontiguousarray(out, dtype=np.float32)



# revision 2
# speedup vs baseline: 1.2415x; 1.2415x over previous
"""Cy2MixerBlock (nn_Cy2MixerBlock_6700148982551) Trainium2 Bass kernel, v2.

Per (b,t) slice (N=325 nodes, D=F=128 features), fully independent:
  natural  = (node partitions [3 tiles 128/128/69], feature free)
  f-major  = (feature partitions [exactly 128], node free [326 padded])
All matmuls in bf16 (moving dims >=256 where possible); LN stats in fp32 via
batched bn_stats; rstd via DVE reciprocal + ACT Sqrt; SGU conv as 9
accumulated matmuls over column-shifted views of a zero-padded vn tile.
8-stage software pipeline over the 48 (b,t) slices per core; output DMAs on
the SP queue so they never block the Activation engine.
"""

import numpy as np
import ml_dtypes
from contextlib import ExitStack

import concourse.bass as bass
import concourse.bacc as bacc
import concourse.mybir as mybir
import concourse.tile as tile
from concourse import bass_utils
from concourse.masks import make_identity

F32 = mybir.dt.float32
F32R = mybir.dt.float32r
BF16 = mybir.dt.bfloat16
AF = mybir.ActivationFunctionType
ALU = mybir.AluOpType

B, T, N, D = 32, 12, 325, 128
F = D
NCORES = 8
BPC = B // NCORES  # batches per core
PKS = (128, 128, 69)  # node-tile partition sizes
NP = 326  # node free dim padded even
EPS = 1e-5
NBF = ml_dtypes.bfloat16


# ---------------------------------------------------------------- host folds
def host_fold(inp):
    """Exact algebraic folds (numpy fp32) -> bf16 device weights."""
    g = inp["norm_g"].astype(np.float32)
    bn = inp["norm_b"].astype(np.float32)
    Wqkv = inp["Wqkv"].astype(np.float32)
    bqkv = inp["bqkv"].astype(np.float32)
    # LN1 affine folded into all consumers of xn
    Wqkv_f = g[:, None] * Wqkv
    bqkv_f = bqkv + bn @ Wqkv
    W1e = inp["W1"].astype(np.float32) + inp["Wl"].astype(np.float32) @ inp["aff1_W"].astype(np.float32)
    b1e = inp["b1"].astype(np.float32) + inp["bl"].astype(np.float32) @ inp["aff1_W"].astype(np.float32) + inp["aff1_b"].astype(np.float32)
    W1f = g[:, None] * W1e
    b1f = b1e + bn @ W1e

    Wq = Wqkv_f[:, 0:F]
    Wk = Wqkv_f[:, F:2 * F]
    Wv = Wqkv_f[:, 2 * F:3 * F]
    A = (Wq @ Wk.T)  # logits = xn A xn^T
    bq = bqkv_f[0:F]
    bk = bqkv_f[F:2 * F]
    bv = bqkv_f[2 * F:3 * F]

    Wo = inp["Wo"].astype(np.float32)
    bo2 = inp["bo"].astype(np.float32) + bv @ Wo  # v-bias exact fold through softmax

    conv_w = inp["conv_w"].astype(np.float32)  # (N, N, 1, K)
    conv_b = inp["conv_b"].astype(np.float32)
    wdtT = np.zeros((3, 3, 128, NP), np.float32)
    for kw in range(3):
        for k in range(3):
            pk = PKS[k]
            wdtT[kw, k, :pk, :N] = conv_w[:, k * 128:k * 128 + pk, 0, kw].T

    cb_const = bool(np.all(conv_b == conv_b[0]))

    d = dict(
        A_bf=A.astype(NBF),
        wv_bf=Wv.astype(NBF),
        w1u_bf=W1f[:, 0:F].astype(NBF),
        w1v_bf=W1f[:, F:2 * F].astype(NBF),
        wo_bf=Wo.astype(NBF),
        w2_bf=inp["W2"].astype(np.float32).astype(NBF),
        wdtT_bf=wdtT.astype(NBF),
        ones_row=np.ones((1, 128), np.float32),
        ones_col_bf=np.ones((128, 1), NBF),
    )
    flags = dict(
        has_bqk=not (np.all(bq == 0) and np.all(bk == 0)),
        has_b1=not np.all(b1f == 0),
        has_bo2=not np.all(bo2 == 0),
        has_b2=not np.all(inp["b2"] == 0),
        has_sgu=not (np.all(inp["sgu_g"] == 1) and np.all(inp["sgu_b"] == 0)),
        cb_const=cb_const,
    )
    extras = dict(
        b1f=b1f, bo2=bo2, b2=inp["b2"].astype(np.float32),
        cb=np.pad(conv_b, (0, NP - N))[None, :].astype(np.float32),
        cb_val=float(conv_b[0]),
        sgu_g=inp["sgu_g"].astype(np.float32), sgu_b=inp["sgu_b"].astype(np.float32),
    )
    return d, flags, extras


# ---------------------------------------------------------------- builder
def build_nc(flags, dbg=False, bpc=BPC, loop_n=None):
    nc = bacc.Bacc("TRN2", target_bir_lowering=False, debug=False)

    x_dram = nc.dram_tensor("x_sh", [bpc, T, N, D], F32, kind="ExternalInput")
    o_dram = nc.dram_tensor("out_sh", [bpc, T, N, D], F32, kind="ExternalOutput")
    wd = {}
    for nm, shp in [("A_bf", [D, D]), ("wv_bf", [D, F]), ("w1u_bf", [D, F]),
                    ("w1v_bf", [D, F]), ("wo_bf", [F, F]), ("w2_bf", [F, D]),
                    ("ones_col_bf", [128, 1])]:
        wd[nm] = nc.dram_tensor(nm, shp, BF16, kind="ExternalInput")
    wd["wdtT_bf"] = nc.dram_tensor("wdtT_bf", [3, 3, 128, NP], BF16, kind="ExternalInput")
    wd["ones_row"] = nc.dram_tensor("ones_row", [1, 128], F32R, kind="ExternalInput")
    if not flags["cb_const"]:
        wd["cb"] = nc.dram_tensor("cb", [1, NP], F32R, kind="ExternalInput")
    if flags["has_b1"]:
        wd["b1fT"] = nc.dram_tensor("b1fT", [2 * F, 1], F32, kind="ExternalInput")
        wd["b1vB"] = nc.dram_tensor("b1vB", [128, F], F32, kind="ExternalInput")
    if flags["has_bo2"] or not flags["cb_const"]:
        # sgbT = bo2 (+ cb_val if cb_const): per-partition additive for sg
        wd["sgbT"] = nc.dram_tensor("sgbT", [F, 1], F32, kind="ExternalInput")
    if flags["has_b2"]:
        wd["b2T"] = nc.dram_tensor("b2T", [D, 1], F32, kind="ExternalInput")
    if flags["has_sgu"]:
        wd["gsB"] = nc.dram_tensor("gsB", [128, F], F32, kind="ExternalInput")
        wd["bsB"] = nc.dram_tensor("bsB", [128, F], F32, kind="ExternalInput")
    assert not flags["has_bqk"], "bq/bk general path not implemented"

    x_ap = x_dram.ap()
    o_ap = o_dram.ap()

    with tile.TileContext(nc) as tc:
        with ExitStack() as ctx:
            const = ctx.enter_context(tc.tile_pool(name="const", bufs=1))
            wrk = ctx.enter_context(tc.tile_pool(name="wrk", bufs=3))
            ps8 = ctx.enter_context(tc.tile_pool(name="ps8", bufs=6, space="PSUM"))
            psz = ctx.enter_context(tc.tile_pool(name="psz", bufs=2, space="PSUM"))

            # ---- constants
            identb = const.tile([128, 128], BF16, name="identb")
            make_identity(nc, identb)
            csb = {}
            for nm, t_ in wd.items():
                if nm == "wdtT_bf":
                    csb[nm] = const.tile([128, 3, 3, NP], BF16, name="c_wdtT")
                    nc.sync.dma_start(out=csb[nm], in_=t_.ap().rearrange("a b p n -> p a b n"))
                    continue
                shp = list(t_.shape)
                csb[nm] = const.tile(shp, t_.dtype, name=f"c_{nm}")
                nc.sync.dma_start(out=csb[nm], in_=t_.ap())

            A_sb = csb["A_bf"]
            wv_sb, w1u_sb, w1v_sb = csb["wv_bf"], csb["w1u_bf"], csb["w1v_bf"]
            wo_sb, w2_sb, wdtT_sb = csb["wo_bf"], csb["w2_bf"], csb["wdtT_bf"]
            ones_row, ones_col = csb["ones_row"], csb["ones_col_bf"]

            # ---- persistent rotating buffers
            def mkbufs(n, shp, dt, name):
                return [const.tile(shp, dt, name=f"{name}{i}") for i in range(n)]

            xts = mkbufs(10, [128, 3, 128], F32, "xt")
            for xb in xts:  # zero never-written region (read by resid add)
                nc.vector.memset(xb[64:128, 2, :], 0.0)
            XnTs = mkbufs(6, [128, NP], BF16, "XnT")
            for xb in XnTs:
                nc.vector.memset(xb[:, 325:326], 0.0)
            Es = mkbufs(4, [128, 3, NP], BF16, "E")
            vnss = mkbufs(4, [128, 3, 128], BF16, "vns")
            zbs = mkbufs(3, [128, NP], F32, "zb")
            atts = mkbufs(5, [128, NP], BF16, "att")
            us = mkbufs(3, [128, NP], BF16, "u")
            vvns = mkbufs(3, [128, 3, 128], BF16, "vvn")
            mv1s = mkbufs(3, [128, 3, 3], F32, "mv1")
            mv2s = mkbufs(3, [128, 3, 3], F32, "mv2")
            for mb in mv1s + mv2s:
                nc.vector.memset(mb, 1.0)
            vn_pads = mkbufs(3, [128, 3, 130], BF16, "vnp")
            for vb in vn_pads:
                nc.vector.memset(vb[:, :, 0:130:129], 0.0)

            def ts_(k):
                return slice(k * 128, k * 128 + PKS[k])

            MAGIC = 0x5F3759DF

            def rsqrt_cols(var_ap, n, eng, tag):
                """rstd = (var+EPS)^-1/2, bit-trick seed (int ops, DVE only)
                + 1 Newton iter on `eng` (float ops; GPSIMD-capable).
                rel err ~3e-6."""
                t_ = wrk.tile([128, 4, 3], F32, tag=tag, bufs=3, name=tag)
                t = t_[:, 0, 0:n]
                y = t_[:, 1, 0:n]
                a = t_[:, 2, 0:n]
                c = t_[:, 3, 0:n]
                nc.vector.tensor_scalar_add(t, var_ap, EPS)
                nc.vector.tensor_scalar(
                    out=y.bitcast(mybir.dt.int32), in0=t.bitcast(mybir.dt.int32),
                    scalar1=1, scalar2=None, op0=ALU.logical_shift_right)
                nc.vector.tensor_scalar(
                    out=y.bitcast(mybir.dt.int32), in0=y.bitcast(mybir.dt.int32),
                    scalar1=-1, scalar2=MAGIC, op0=ALU.mult, op1=ALU.add)
                eng.tensor_mul(a, y, y)
                eng.tensor_mul(a, a, t)
                eng.tensor_scalar(out=c, in0=a, scalar1=-0.5, scalar2=1.5,
                                  op0=ALU.mult, op1=ALU.add)
                eng.tensor_mul(y, y, c)
                return y

            if flags["has_bo2"] or not flags["cb_const"]:
                sg_scalar = csb["sgbT"]
            else:
                sg_scalar = extras_cb_val[0]

            S = {}

            def s0(b, t):
                st = S[(b, t)] = {}
                i = b * T + t
                xt = xts[i % 10]
                nc.sync.dma_start(
                    out=xt[:, 0:2, :],
                    in_=x_ap[b, t, 0:256, :].rearrange("(k p) d -> p k d", p=128))
                nc.sync.dma_start(out=xt[:69, 2, :], in_=x_ap[b, t, 256:325, :])
                st["xt"] = xt

            def s1a(b, t):
                st = S[(b, t)]
                i = b * T + t
                xt = st["xt"]
                mv = mv1s[i % 3]
                stx = wrk.tile([128, 3, 6], F32, tag="stx", bufs=2, name="stx")
                for k in range(3):
                    pk = PKS[k]
                    nc.vector.bn_stats(stx[:pk, k, :], xt[:pk, k, :])
                    nc.vector.bn_aggr(mv[:pk, k, 0:2], stx[:pk, k, :])
                st["mv1"] = mv

            def s1b(b, t):
                st = S[(b, t)]
                i = b * T + t
                rs = rsqrt_cols(st["mv1"][:, :, 1], 3, nc.vector, "rs1")
                XnT = XnTs[i % 6]
                tpx = ps8.tile([128, 384], BF16, tag="ps", name="tpx")
                for k in range(3):
                    pk = PKS[k]
                    xtmp = wrk.tile([128, 128], BF16, tag="xtmp", bufs=3, name="xtmp")
                    nc.gpsimd.tensor_scalar(
                        out=xtmp[:pk], in0=st["xt"][:pk, k, :],
                        scalar1=st["mv1"][:pk, k, 0:1], scalar2=rs[:pk, k:k + 1],
                        op0=ALU.subtract, op1=ALU.mult)
                    nc.tensor.transpose(tpx[:, 128 * k:128 * k + pk],
                                        xtmp[:pk, :], identb[:pk, :pk])
                nc.scalar.copy(out=XnT[:, 0:325], in_=tpx[:, 0:325])
                st["XnT"] = XnT

            def s2a(b, t):
                st = S[(b, t)]
                i = b * T + t
                XnT = st["XnT"]
                g_ps = ps8.tile([128, NP], F32, tag="ps", name="g_ps")
                nc.tensor.matmul(g_ps, A_sb, XnT, start=True, stop=True)
                G = wrk.tile([128, NP], BF16, tag="G", bufs=3, name="G")
                nc.scalar.copy(G, g_ps)
                E = Es[i % 4]
                for k in range(3):
                    pk = PKS[k]
                    lt = ps8.tile([128, NP], F32, tag="ps", name="lt")
                    nc.tensor.matmul(lt[:pk], XnT[:, ts_(k)], G, start=True, stop=True)
                    nc.scalar.activation(E[:pk, k, :], lt[:pk], AF.Exp)
                vp = ps8.tile([128, 384], F32, tag="ps", name="vp")
                for k in range(3):
                    pk = PKS[k]
                    nc.tensor.matmul(vp[:pk, 128 * k:128 * (k + 1)],
                                     XnT[:, ts_(k)], wv_sb, start=True, stop=True)
                vns = vnss[i % 4]
                nc.scalar.copy(out=vns, in_=vp)
                st["E"] = E
                st["vns"] = vns

            def s2z(b, t):
                st = S[(b, t)]
                E = st["E"]
                zzb = psz.tile([128, NP], F32, tag="psz", name="zzb")
                for k in range(3):
                    nc.tensor.matmul(zzb[0:1, :], ones_col[:PKS[k]], E[:PKS[k], k, :],
                                     start=(k == 0), stop=(k == 2))
                st["zzb"] = zzb

            def s2zz(b, t):
                st = S[(b, t)]
                i = b * T + t
                zzb = st["zzb"]
                zr = wrk.tile([1, NP], F32R, tag="zr", bufs=3, name="zr")
                with nc.allow_low_precision(reason="f32r rounding of 1/Z is fine"):
                    nc.vector.reciprocal(zr, zzb[0:1, :])
                nc.tensor.matmul(zzb, ones_row, zr, start=True, stop=True)
                zbt = zbs[i % 3]
                nc.vector.tensor_copy(zbt, zzb)
                st["zb"] = zbt

            def s2att(b, t):
                st = S[(b, t)]
                i = b * T + t
                au = ps8.tile([128, NP], F32, tag="ps", name="au")
                for k in range(3):
                    pk = PKS[k]
                    nc.tensor.matmul(au, st["vns"][:pk, k, :], st["E"][:pk, k, :],
                                     start=(k == 0), stop=(k == 2))
                att = atts[i % 5]
                nc.vector.tensor_mul(att, au, st["zb"])
                st["att"] = att

            def s2b(b, t):
                st = S[(b, t)]
                i = b * T + t
                XnT = st["XnT"]
                up = ps8.tile([128, NP], F32, tag="ps", name="up")
                nc.tensor.matmul(up, w1u_sb, XnT, start=True, stop=True)
                u = us[i % 3]
                ub = csb["b1fT"][0:F] if flags["has_b1"] else 0.0
                nc.scalar.activation(u, up, AF.Relu, bias=ub)
                vvp = ps8.tile([128, 384], F32, tag="ps", name="vvp")
                for k in range(3):
                    pk = PKS[k]
                    nc.tensor.matmul(vvp[:pk, 128 * k:128 * (k + 1)],
                                     XnT[:, ts_(k)], w1v_sb, start=True, stop=True)
                vvn = vvns[i % 3]
                if flags["has_b1"]:
                    for k in range(3):
                        pk = PKS[k]
                        nc.vector.tensor_add(vvn[:pk, k, :],
                                             vvp[:pk, 128 * k:128 * (k + 1)],
                                             csb["b1vB"][:pk])
                        nc.vector.tensor_scalar_max(vvn[:pk, k, :], vvn[:pk, k, :], 0.0)
                else:
                    nc.scalar.activation(vvn[:, 0:2, :], vvp[:, 0:256], AF.Relu)
                    nc.scalar.activation(vvn[:69, 2, :], vvp[:69, 256:384], AF.Relu)
                mv = mv2s[i % 3]
                stx = wrk.tile([128, 3, 6], F32, tag="stx2", bufs=2, name="stx2")
                for k in range(3):
                    pk = PKS[k]
                    nc.vector.bn_stats(stx[:pk, k, :], vvn[:pk, k, :])
                    nc.vector.bn_aggr(mv[:pk, k, 0:2], stx[:pk, k, :])
                st["u"] = u
                st["vvn"] = vvn
                st["mv2"] = mv

            def s2c(b, t):
                st = S[(b, t)]
                i = b * T + t
                rs = rsqrt_cols(st["mv2"][:, :, 1], 3, nc.vector, "rs2")
                vn = vn_pads[i % 3]
                for k in range(3):
                    pk = PKS[k]
                    nc.gpsimd.tensor_scalar(
                        out=vn[:pk, k, 1:129], in0=st["vvn"][:pk, k, :],
                        scalar1=st["mv2"][:pk, k, 0:1], scalar2=rs[:pk, k:k + 1],
                        op0=ALU.subtract, op1=ALU.mult)
                    if flags["has_sgu"]:
                        nc.vector.tensor_mul(vn[:pk, k, 1:129], vn[:pk, k, 1:129], csb["gsB"][:pk])
                        nc.vector.tensor_add(vn[:pk, k, 1:129], vn[:pk, k, 1:129], csb["bsB"][:pk])
                st["vn"] = vn

            def s3(b, t):
                st = S[(b, t)]
                co = ps8.tile([128, NP], F32, tag="ps", name="co")
                first = True
                for kw in range(3):
                    for k in range(3):
                        pk = PKS[k]
                        nc.tensor.matmul(co, st["vn"][:pk, k, kw:kw + 128],
                                         wdtT_sb[:pk, kw, k, :],
                                         start=first, stop=False)
                        first = False
                if not flags["cb_const"]:
                    nc.tensor.matmul(co, ones_row, csb["cb"], start=False, stop=False)
                nc.tensor.matmul(co, wo_sb, st["att"], start=False, stop=True)
                sg = wrk.tile([128, NP], BF16, tag="sg", bufs=2, name="sg")
                nc.vector.scalar_tensor_tensor(out=sg, in0=co, scalar=sg_scalar,
                                               in1=st["u"], op0=ALU.add, op1=ALU.mult)
                op_ = ps8.tile([128, NP], F32, tag="ps", name="op_")
                nc.tensor.matmul(op_, w2_sb, sg, start=True, stop=True)
                ot = wrk.tile([128, NP], BF16, tag="ot", bufs=2, name="ot")
                b2b = csb["b2T"] if flags["has_b2"] else 0.0
                nc.scalar.activation(ot, op_, AF.Relu, bias=b2b)
                tpo = ps8.tile([128, 3, 128], BF16, tag="ps", name="tpo")
                for k in range(3):
                    pk = PKS[k]
                    nc.tensor.transpose(tpo[:pk, k, :], ot[:, ts_(k)], identb)
                onat = wrk.tile([128, 3, 128], F32, tag="onat", bufs=2, name="onat")
                nc.vector.tensor_add(onat, tpo, st["xt"])
                nc.sync.dma_start(
                    out=o_ap[b, t, 0:256, :].rearrange("(k p) d -> p k d", p=128),
                    in_=onat[:, 0:2, :])
                nc.sync.dma_start(out=o_ap[b, t, 256:325, :], in_=onat[:69, 2, :])

            seq = [(b, t) for b in range(bpc) for t in range(T)]
            STAGES = [s0, s1a, s1b, s2a, s2z, s2zz, s2att, s2b, s2c, s3]
            NS = len(STAGES)

            def emit_all():
                S.clear()
                for i in range(len(seq) + NS - 1):
                    for j in reversed(range(NS)):
                        k = i - j
                        if 0 <= k < len(seq):
                            STAGES[j](*seq[k])
                    if i - (NS - 1) >= 0:
                        del S[seq[i - (NS - 1)]]

            if loop_n:
                with tc.For_i(0, loop_n, 1):
                    emit_all()
            else:
                emit_all()

    nc.compile()
    return nc


# stand-in so build_nc can reference the const-conv-bias value; set per-call
extras_cb_val = [1.0]


# ---------------------------------------------------------------- runner
def make_in_maps(inputs, dev, flags, extras):
    x = np.ascontiguousarray(inputs["x"], dtype=np.float32)
    maps = []
    for c in range(NCORES):
        m = dict(dev)
        if not flags["cb_const"]:
            m["cb"] = extras["cb"]
        if flags["has_b1"]:
            m["b1fT"] = extras["b1f"][:, None].astype(np.float32)
            m["b1vB"] = np.broadcast_to(extras["b1f"][None, F:2 * F], (128, F)).astype(np.float32).copy()
        if flags["has_bo2"] or not flags["cb_const"]:
            sgb = extras["bo2"].astype(np.float32).copy()
            if flags["cb_const"]:
                sgb = sgb + extras["cb_val"]
            m["sgbT"] = sgb[:, None]
        if flags["has_b2"]:
            m["b2T"] = extras["b2"][:, None].astype(np.float32)
        if flags["has_sgu"]:
            m["gsB"] = np.broadcast_to(extras["sgu_g"][None, :], (128, F)).astype(np.float32).copy()
            m["bsB"] = np.broadcast_to(extras["sgu_b"][None, :], (128, F)).astype(np.float32).copy()
        m["x_sh"] = x[c * BPC:(c + 1) * BPC]
        maps.append(m)
    return maps


_NC_CACHE = {}


def kernel(**inputs):
    dev, flags, extras = host_fold(inputs)
    key = tuple(sorted(flags.items()))
    if flags["cb_const"]:
        key = key + (("cb_val", extras["cb_val"]),)
    if key not in _NC_CACHE:
        extras_cb_val[0] = extras["cb_val"]
        _NC_CACHE[key] = build_nc(flags, dbg=False)
    nc = _NC_CACHE[key]
    in_maps = make_in_maps(inputs, dev, flags, extras)
    res = bass_utils.run_bass_kernel_spmd(nc, in_maps, core_ids=list(range(NCORES)))
    out = np.concatenate([res.results[c]["out_sh"] for c in range(NCORES)], axis=0)
    return np.ascontiguousarray(out, dtype=np.float32)
